# revision 10
# baseline (speedup 1.0000x reference)
"""Multi-head attention kernel for Trainium2, tensor-parallel over heads on 8 cores.

Strategy (per core c, heads [2c, 2c+1]):
  - host feeds X^T [D, B*S] (shared), per-core transposed head weights, and the
    matching Wo column-slice; each core computes a full-shape partial of the
    output projection, host sums the 8 partials and adds bo.
  - on device everything is computed in "transposed" orientation so every
    matmul contracts over the partition dim with no on-device transposes of
    activations (only V needs a PE-transpose):
      QT/KT/VT [e, s] = W @ X^T          (fp32r matmuls, N=512)
      S^T [t, s]      = KT.T @ QT        (per (b, head), C=64)
      P^T             = exp(S^T / 8)     (ACT, straight from PSUM)
      [avT ; l]       = [V | 1].T @ P^T  (fused unnormalized attention + sum)
      Z               = avT * (1/l)      (DVE, broadcast via DRAM bounce)
      out_partial     = Z.T @ WoT_slice  (PSUM -> DRAM)
"""

import numpy as np

import concourse.bass as bass
import concourse.mybir as mybir
import concourse.tile as tile
from concourse import bacc
from concourse.bass_utils import run_bass_kernel_spmd
from concourse.masks import make_identity

# Problem shapes (hardcoded per contract).
B, S, D = 4, 2048, 1024
H, E = 16, 64
NCORES = 8
HPC = H // NCORES          # heads per core = 2
EC = HPC * E               # per-core head width = 128
BS = B * S                 # 8192 rows
P = 128
DC = D // P                # 8 contraction chunks for the projections
ST = 512                   # s tile (matmul moving free dim)
N_ST = S // ST             # 4 s-tiles per batch
TCH = S // P               # 16 key chunks per batch

F32 = mybir.dt.float32
F32R = mybir.dt.float32r
EXP = mybir.ActivationFunctionType.Exp


def _r(ap):
    return ap.bitcast(F32R)


def build_module():
    """Build the single-core Bass module (same NEFF runs SPMD on all 8 cores)."""
    from contextlib import ExitStack

    nc = bacc.Bacc("TRN2", target_bir_lowering=False, debug=False)
    xt = nc.dram_tensor("xt", [D, BS], F32, kind="ExternalInput").ap()
    wq = nc.dram_tensor("wq_t", [D, EC], F32, kind="ExternalInput").ap()
    wk = nc.dram_tensor("wk_t", [D, EC], F32, kind="ExternalInput").ap()
    wv = nc.dram_tensor("wv_t", [D, EC], F32, kind="ExternalInput").ap()
    bq = nc.dram_tensor("bq", [EC, 1], F32, kind="ExternalInput").ap()
    bk = nc.dram_tensor("bk", [EC, 1], F32, kind="ExternalInput").ap()
    bv = nc.dram_tensor("bv", [EC, 1], F32, kind="ExternalInput").ap()
    wo = nc.dram_tensor("wo_t", [EC, D], F32, kind="ExternalInput").ap()
    outp = nc.dram_tensor("out_p", [BS, D], F32, kind="ExternalOutput").ap()

    xt_r = xt.rearrange("(dc p) s -> p dc s", p=P)    # [128, 8, 8192]
    wq_r = wq.rearrange("(dc p) e -> p dc e", p=P)    # [128, 8, 128]
    wk_r = wk.rearrange("(dc p) e -> p dc e", p=P)
    wv_r = wv.rearrange("(dc p) e -> p dc e", p=P)

    with tile.TileContext(nc) as tc, ExitStack() as ctx:
        singles = ctx.enter_context(tc.tile_pool(name="singles", bufs=1))

        wq_sb = singles.tile([P, DC, EC], F32R, tag="wq")
        wk_sb = singles.tile([P, DC, EC], F32R, tag="wk")
        wv_sb = singles.tile([P, DC, EC], F32R, tag="wv")
        nc.sync.dma_start(wq_sb[:], _r(wq_r))
        nc.sync.dma_start(wk_sb[:], _r(wk_r))
        nc.sync.dma_start(wv_sb[:], _r(wv_r))
        bq_sb = singles.tile([EC, 1], F32, tag="bq")
        bk_sb = singles.tile([EC, 1], F32, tag="bk")
        bv_sb = singles.tile([EC, 1], F32, tag="bv")
        nc.sync.dma_start(bq_sb[:], bq)
        nc.sync.dma_start(bk_sb[:], bk)
        nc.sync.dma_start(bv_sb[:], bv)
        wo_sb = singles.tile([EC, D], F32R, tag="wo")
        nc.sync.dma_start(wo_sb[:], _r(wo))
        ident = singles.tile([P, P], F32, tag="ident")
        make_identity(nc, ident[:])

        # Per-batch persistent activations: [e, s] projections and V_ext.
        qt = [singles.tile([EC, S], F32R, tag=f"qt{b}", name=f"qt{b}") for b in range(B)]
        kt = [singles.tile([EC, S], F32R, tag=f"kt{b}", name=f"kt{b}") for b in range(B)]
        vt = [singles.tile([EC, S], F32, tag=f"vtz{b}", name=f"vt{b}") for b in range(B)]
        # V_ext layout: [t-part, t-chunk, 130] = [V_h0 | 1 | V_h1 | 1]
        vx = [singles.tile([P, TCH, 2 * E + 2], F32R, tag=f"vx{b}", name=f"vx{b}") for b in range(B)]
        ones_f32 = singles.tile([P, 1], F32, tag="ones")
        nc.vector.memset(ones_f32[:], 1.0)
        for b in range(B):
            nc.vector.tensor_copy(
                vx[b][:, :, E : E + 1], ones_f32.to_broadcast((P, TCH, 1))
            )
            nc.vector.tensor_copy(
                vx[b][:, :, 2 * E + 1 : 2 * E + 2], ones_f32.to_broadcast((P, TCH, 1))
            )

        # ---- Phase 1+2: projections and V transpose --------------------------
        with (
            tc.tile_pool(name="xload", bufs=3) as xpool,
            tc.tile_pool(name="ps12", bufs=2, space="PSUM") as ps12,
        ):
            for g in range(BS // ST):                 # 16 global s-tiles
                b, st = g // N_ST, g % N_ST
                sl = slice(st * ST, (st + 1) * ST)
                x_t = xpool.tile([P, DC, ST], F32R, tag="xt")
                nc.sync.dma_start(x_t[:], _r(xt_r[:, :, g * ST : (g + 1) * ST]))
                for w_sb, b_sb, dst in (
                    (wq_sb, bq_sb, qt[b]),
                    (wk_sb, bk_sb, kt[b]),
                    (wv_sb, bv_sb, vt[b]),
                ):
                    ps = ps12.tile([P, ST], F32, tag="proj")
                    for dc in range(DC):
                        nc.tensor.matmul(
                            ps[:],
                            w_sb[:, dc],
                            x_t[:, dc],
                            start=(dc == 0),
                            stop=(dc == DC - 1),
                        )
                    nc.vector.tensor_scalar_add(dst[:, sl], ps[:], b_sb[:])
                # PE-transpose this s-tile's V chunks into V_ext.
                for c in range(ST // P):
                    tch = st * (ST // P) + c
                    tp = ps12.tile([P, P], F32, tag="tp")
                    nc.tensor.transpose(
                        tp[:], vt[b][:, tch * P : (tch + 1) * P], ident[:]
                    )
                    nc.vector.tensor_copy(vx[b][:, tch, 0:E], tp[:, 0:E])
                    nc.vector.tensor_copy(
                        vx[b][:, tch, E + 1 : 2 * E + 1], tp[:, E : 2 * E]
                    )

        # ---- Phase 3+4: attention + output projection ------------------------
        z = [singles.tile([EC, S], F32R, tag=f"vtz{b}", name=f"z{b}") for b in range(B)]  # alias vt
        with (
            tc.tile_pool(name="pexp", bufs=4) as ppool,
            tc.tile_pool(name="bcast", bufs=2) as bpool,
            tc.tile_pool(name="ostage", bufs=3) as opool,
            tc.tile_pool(name="lrow", bufs=2, space="DRAM") as dpool,
            tc.tile_pool(name="ps_sc", bufs=2, space="PSUM") as ps_sc,
            tc.tile_pool(name="ps_av", bufs=1, space="PSUM") as ps_av,
            tc.tile_pool(name="ps_o", bufs=2, space="PSUM") as ps_o,
        ):
            for b in range(B):
                for st in range(N_ST):
                    ssl = slice(st * ST, (st + 1) * ST)
                    av0 = ps_av.tile([P, ST], F32, tag="av0")
                    av1 = ps_av.tile([P, ST], F32, tag="av1")
                    for t in range(TCH):
                        tsl = slice(t * P, (t + 1) * P)
                        sc = ps_sc.tile([P, 2, ST], F32, tag="sc")
                        nc.tensor.matmul(
                            sc[:, 0], kt[b][0:E, tsl], qt[b][0:E, ssl],
                            start=True, stop=True,
                        )
                        nc.tensor.matmul(
                            sc[:, 1], kt[b][E : 2 * E, tsl], qt[b][E : 2 * E, ssl],
                            start=True, stop=True,
                        )
                        pt = ppool.tile([P, 2, ST], F32R, tag="pt")
                        nc.scalar.activation(pt[:], sc[:], EXP, scale=0.125)
                        nc.tensor.matmul(
                            av0[0 : E + 1], vx[b][:, t, 0 : E + 1], pt[:, 0],
                            start=(t == 0), stop=(t == TCH - 1),
                        )
                        nc.tensor.matmul(
                            av1[0 : E + 1], vx[b][:, t, E + 1 : 2 * E + 2], pt[:, 1],
                            start=(t == 0), stop=(t == TCH - 1),
                        )
                    # Unnormalized copy out of PSUM (frees the av banks fast),
                    # reciprocal of the fused row-sums, broadcast via DRAM.
                    nc.vector.tensor_copy(z[b][0:E, ssl], av0[0:E])
                    nc.vector.tensor_copy(z[b][E : 2 * E, ssl], av1[0:E])
                    lr0 = bpool.tile([1, ST], F32, tag="lr0")
                    lr1 = bpool.tile([1, ST], F32, tag="lr1")
                    nc.vector.reciprocal(lr0[0:1], av0[E : E + 1])
                    nc.vector.reciprocal(lr1[0:1], av1[E : E + 1])
                    lrow = dpool.tile([2, ST], F32, tag="lrow")
                    nc.sync.dma_start(lrow[0:1], lr0[0:1])
                    nc.sync.dma_start(lrow[1:2], lr1[0:1])
                    bc = bpool.tile([P, ST], F32, tag="bc")
                    nc.sync.dma_start(
                        bc[0:E],
                        bass.AP(tensor=lrow.tensor, offset=lrow.offset,
                                ap=[[0, E]] + list(lrow[0, :].ap)),
                    )
                    nc.sync.dma_start(
                        bc[E : 2 * E],
                        bass.AP(tensor=lrow.tensor, offset=lrow.offset + ST,
                                ap=[[0, E]] + list(lrow[1, :].ap)),
                    )
                    nc.vector.tensor_mul(z[b][0:E, ssl], z[b][0:E, ssl], bc[0:E])
                    nc.vector.tensor_mul(
                        z[b][E : 2 * E, ssl], z[b][E : 2 * E, ssl], bc[E : 2 * E]
                    )
                    # Output projection for this s-tile's four 128-row chunks.
                    for c in range(ST // P):
                        zsl = slice(st * ST + c * P, st * ST + (c + 1) * P)
                        rows = slice(b * S + st * ST + c * P, b * S + st * ST + (c + 1) * P)
                        for oh in range(D // 512):
                            po = ps_o.tile([P, 512], F32, tag="po")
                            nc.tensor.matmul(
                                po[:], z[b][:, zsl], wo_sb[:, oh * 512 : (oh + 1) * 512],
                                start=True, stop=True,
                            )
                            osb = opool.tile([P, 512], F32, tag="osb")
                            nc.vector.tensor_copy(osb[:], po[:])
                            nc.sync.dma_start(outp[rows, oh * 512 : (oh + 1) * 512], osb[:])
    nc.finalize()
    return nc


_NC_CACHE = None


def _get_module():
    global _NC_CACHE
    if _NC_CACHE is None:
        _NC_CACHE = build_module()
    return _NC_CACHE


def prepare_in_maps(inputs):
    x = np.ascontiguousarray(np.asarray(inputs["input_matrix"], np.float32))
    wq = np.asarray(inputs["Wq"], np.float32)
    wk = np.asarray(inputs["Wk"], np.float32)
    wv = np.asarray(inputs["Wv"], np.float32)
    bq = np.asarray(inputs["bq"], np.float32)
    bk = np.asarray(inputs["bk"], np.float32)
    bv = np.asarray(inputs["bv"], np.float32)
    wo = np.asarray(inputs["Wo"], np.float32)

    xt = np.ascontiguousarray(x.reshape(BS, D).T)            # [D, BS]
    in_maps = []
    for c in range(NCORES):
        hs = slice(HPC * c, HPC * (c + 1))
        m = {
            "xt": xt,
            "wq_t": np.ascontiguousarray(wq[hs].transpose(2, 0, 1).reshape(D, EC)),
            "wk_t": np.ascontiguousarray(wk[hs].transpose(2, 0, 1).reshape(D, EC)),
            "wv_t": np.ascontiguousarray(wv[hs].transpose(2, 0, 1).reshape(D, EC)),
            "bq": np.ascontiguousarray(bq[hs].reshape(EC, 1)),
            "bk": np.ascontiguousarray(bk[hs].reshape(EC, 1)),
            "bv": np.ascontiguousarray(bv[hs].reshape(EC, 1)),
            "wo_t": np.ascontiguousarray(wo[:, EC * c : EC * (c + 1)].T),
        }
        in_maps.append(m)
    return in_maps


def finish(results, inputs):
    bo = np.asarray(inputs["bo"], np.float32)
    acc = results[0]["out_p"].astype(np.float64)
    for r in results[1:]:
        acc += r["out_p"]
    out = (acc + bo).astype(np.float32)
    return out.reshape(B, S, D)


def kernel(**inputs):
    nc = _get_module()
    in_maps = prepare_in_maps(inputs)
    res = run_bass_kernel_spmd(nc, in_maps, core_ids=list(range(NCORES)))
    return finish(res.results, inputs)


if __name__ == "__main__":
    import reference

    inputs = {k: np.asarray(v) for k, v in reference.setup_inputs().items()}
    out = kernel(**inputs)
    print(out.shape, out.dtype)
